# revision 19
# baseline (speedup 1.0000x reference)
"""Steady-state diffusion-degradation morphogen field kernel for Trainium2.

Computes, for every cell i and morphogen m:
    conc[i, m] = sum_j G_m(r_ij) * secretion[j, m] * active[j]
with G_m(r) = exp(-r / lambda_m) / (4 pi D_m r), lambda_m = sqrt(D_m / k_m),
r_ij = max(|p_i - p_j|, radius_j).

Strategy (8 NeuronCores, data-parallel over query rows i):
  * Each core owns 512 query rows; all 4096 sources are replicated.
  * dist^2 via one K=5 augmented matmul per 128-source block:
      s[j, i] = x_j*(-2x_i) + y_j*(-2y_i) + z_j*(-2z_i) + 1*|p_i|^2 + |p_j|^2*1
    with per-source-block local centering (cells Morton-sorted) so the
    cancellation error stays at the scale of the actual distances.
  * One ACT table set (natural_log_exp) does all transcendentals:
      L = ln(max(s, radius_j^2));  r = exp(0.5*L)
      E_g = exp(-(r/lam_g + 0.5*L)) = exp(-r/lam_g)/r     (1/r folded in!)
  * Per distinct lambda the DVE builds the argument with the fused
    affine_then_add op; PE contracts E_g against stationary
    src'[j, m] = secretion*active/(4 pi D_m), accumulating in PSUM.
"""

import os
import sys

import numpy as np

for _p in ("/opt/trn_rl_repo", "/root/.axon_site/_ro/trn_rl_repo"):
    if os.path.isdir(_p) and _p not in sys.path:
        sys.path.append(_p)

N = 4096
M = 8
NCORES = 8
RPC = N // NCORES          # 512 query rows per core
PB = 128                   # source rows per block (partition dim)
NB = N // PB               # 32 source blocks
CHUNK_BLOCKS = 4           # source blocks per elementwise chunk
CHUNK_F = CHUNK_BLOCKS * RPC  # free extent of a chunk tile
FOUR_PI = 4.0 * np.pi

# dtype knobs ("f32", "f32r", "f16", "bf16")
DIST_MM_DT = "f32"         # dist^2 matmul operand dtype
REDUCE_MM_DT = "f32"      # reduction matmul operand dtype (E and src)
GPSIMD_GROUPS = 0          # GpSimd elementwise offload is a net loss (shared
                           # SBUF port starves DVE; its TENSOR_SCALAR is 17cyc/elem)

_compiled = None           # (key, nc) compile cache


def _morton_order(pos):
    """Spatial sort so each 128-cell block is spatially local."""
    span = np.maximum(pos.max(0) - pos.min(0), 1e-30)
    q = np.clip((pos - pos.min(0)) / span * 1023.0, 0, 1023).astype(np.uint64)

    def _spread(v):
        v &= 0x3FF
        v = (v | (v << 16)) & 0x030000FF
        v = (v | (v << 8)) & 0x0300F00F
        v = (v | (v << 4)) & 0x030C30C3
        v = (v | (v << 2)) & 0x09249249
        return v

    code = (_spread(q[:, 0]) << 2) | (_spread(q[:, 1]) << 1) | _spread(q[:, 2])
    return np.argsort(code, kind="stable")


def _build_groups(lam):
    """Group channels by identical fp32 lambda. Returns (lams, perm, offs, ns)."""
    uniq = np.unique(lam)
    chans, lams = [], []
    for u in uniq:
        idx = np.nonzero(lam == u)[0]
        chans.append(idx)
        lams.append(float(u))
    perm = np.concatenate(chans)
    ns = [len(c) for c in chans]
    offs = np.concatenate([[0], np.cumsum(ns)])[:-1].tolist()
    return lams, perm, offs, ns


def _patch_act_tables():
    """Keep Exp/Ln only in natural_log_exp_and_others so the table-load
    inserter picks one set for both (indices must stay act_info-aligned)."""
    from concourse import bacc, mybir

    if getattr(bacc, "_act_tables_patched", False):
        return
    orig = bacc.get_activation_tables

    def patched(arch):
        tabs = orig(arch)
        out = {}
        for name, fns in tabs.items():
            if name != "natural_log_exp_and_others":
                fns = fns - {mybir.ActivationFunctionType.Exp,
                             mybir.ActivationFunctionType.Ln}
            out[name] = fns
        return out

    bacc.get_activation_tables = patched
    bacc._act_tables_patched = True


def _build_program(group_lams, group_offs, group_ns):
    from contextlib import ExitStack

    import concourse.bass as bass
    import concourse.tile as tile
    from concourse import bacc, mybir

    _patch_act_tables()

    f32 = mybir.dt.float32
    mm_dt = {"f32": mybir.dt.float32, "f32r": mybir.dt.float32r,
             "f16": mybir.dt.float16, "bf16": mybir.dt.bfloat16}
    dist_dt = mm_dt[DIST_MM_DT]
    red_dt = mm_dt[REDUCE_MM_DT]
    Exp = mybir.ActivationFunctionType.Exp
    Ln = mybir.ActivationFunctionType.Ln

    nc = bacc.Bacc("TRN2", target_bir_lowering=False, debug=False,
                   enable_asserts=False, num_devices=NCORES)

    aug_src = nc.dram_tensor("aug_src", [5, N], f32, kind="ExternalInput").ap()
    aug_q = nc.dram_tensor("aug_q", [5, NB * RPC], f32, kind="ExternalInput").ap()
    radsq = nc.dram_tensor("radsq", [PB, NB], f32, kind="ExternalInput").ap()
    srct = nc.dram_tensor("srct", [PB, NB * M], red_dt,
                          kind="ExternalInput").ap()
    outT = nc.dram_tensor("outT", [M, RPC], f32, kind="ExternalOutput").ap()

    ngroups = len(group_lams)
    assert ngroups <= 6, "PSUM bank budget supports at most 6 lambda groups"

    with tile.TileContext(nc) as tc, ExitStack() as ctx:
        const = ctx.enter_context(tc.tile_pool(name="const", bufs=1))
        aug_src_s = const.tile([5, N], f32, tag="augsrc")
        nc.sync.dma_start(aug_src_s[:], aug_src[:])
        radsq_s = const.tile([PB, NB], f32, tag="radsq")
        nc.sync.dma_start(radsq_s[:], radsq[:])
        srct_s = const.tile([PB, NB * M], red_dt, tag="srct")
        nc.sync.dma_start(srct_s[:], srct[:])

        ps_s = ctx.enter_context(tc.tile_pool(name="ps_s", bufs=2, space="PSUM"))
        ps_o = ctx.enter_context(tc.tile_pool(name="ps_o", bufs=1, space="PSUM"))
        aq_pool = ctx.enter_context(tc.tile_pool(name="aq", bufs=4))
        sc_pool = ctx.enter_context(tc.tile_pool(name="sc", bufs=2))
        ln_pool = ctx.enter_context(tc.tile_pool(name="lnp", bufs=2))
        r_pool = ctx.enter_context(tc.tile_pool(name="rp", bufs=2))
        a_pool = ctx.enter_context(tc.tile_pool(name="ap", bufs=3))
        g_pool = ctx.enter_context(tc.tile_pool(name="gp", bufs=3))
        e_pool = ctx.enter_context(tc.tile_pool(name="ep", bufs=4))

        ps_out = [ps_o.tile([group_ns[g], RPC], f32, tag=f"out{g}",
                            name=f"ps_out{g}")
                  for g in range(ngroups)]

        nchunks = NB // CHUNK_BLOCKS
        for cc in range(nchunks):
            sc = sc_pool.tile([PB, CHUNK_F], f32, tag="sc")
            for bi in range(CHUNK_BLOCKS):
                b = cc * CHUNK_BLOCKS + bi
                aq_t = aq_pool.tile([5, RPC], f32, tag="aq")
                nc.sync.dma_start(aq_t[:], aug_q[:, b * RPC:(b + 1) * RPC])
                ps_tile = ps_s.tile([PB, RPC], f32, tag="s2")
                nc.tensor.matmul(
                    ps_tile[:],
                    lhsT=aug_src_s[:, b * PB:(b + 1) * PB].bitcast(dist_dt),
                    rhs=aq_t[:].bitcast(dist_dt),
                    start=True, stop=True,
                )
                nc.vector.tensor_scalar_max(
                    sc[:, bi * RPC:(bi + 1) * RPC], ps_tile[:],
                    radsq_s[:, b:b + 1],
                )
            lnt = ln_pool.tile([PB, CHUNK_F], f32, tag="ln")
            nc.scalar.activation(lnt[:], sc[:], Ln)
            rt = r_pool.tile([PB, CHUNK_F], f32, tag="r")
            nc.scalar.activation(rt[:], lnt[:], Exp, scale=0.5)
            # interleave GpSimd-offloaded groups among the DVE ones
            n_gp = min(GPSIMD_GROUPS, ngroups - 1)
            gp_set = set(range(0, ngroups, max(1, ngroups // max(n_gp, 1)))
                         ) if n_gp else set()
            gp_set = set(list(gp_set)[:n_gp])
            for g in range(ngroups):
                lam_g = group_lams[g]
                if g in gp_set:
                    gt = g_pool.tile([PB, CHUNK_F], f32, tag="gt")
                    nc.gpsimd.tensor_scalar_mul(gt[:], lnt[:], lam_g * 0.5)
                    at = g_pool.tile([PB, CHUNK_F], f32, tag="ag")
                    nc.gpsimd.tensor_tensor(at[:], gt[:], rt[:],
                                            mybir.AluOpType.add)
                else:
                    at = a_pool.tile([PB, CHUNK_F], f32, tag="a")
                    nc.vector.affine_then_add(
                        at[:], in0=lnt[:], in1=rt[:], scale=lam_g * 0.5,
                        bias=0.0)
                et = e_pool.tile([PB, CHUNK_F], red_dt, tag="e")
                nc.scalar.activation(et[:], at[:], Exp, scale=-1.0 / lam_g)
                for bi in range(CHUNK_BLOCKS):
                    b = cc * CHUNK_BLOCKS + bi
                    nc.tensor.matmul(
                        ps_out[g][:],
                        lhsT=srct_s[:, b * M + group_offs[g]:
                                    b * M + group_offs[g] + group_ns[g]],
                        rhs=et[:, bi * RPC:(bi + 1) * RPC],
                        start=(b == 0), stop=(b == NB - 1),
                    )

        out_pool = ctx.enter_context(tc.tile_pool(name="outp", bufs=2))
        for g in range(ngroups):
            o = group_offs[g]
            sb = out_pool.tile([3, RPC], f32, tag="osb", name=f"out_sb{g}")
            nc.vector.tensor_copy(sb[0:group_ns[g], :], ps_out[g][:])
            nc.sync.dma_start(outT[o:o + group_ns[g], :], sb[0:group_ns[g], :])

    nc.compile()
    return nc


def _prepare(position, radius, secretion, diffusion_coefs, degradation_rates,
             active):
    pos = np.asarray(position, np.float64)
    rad = np.asarray(radius, np.float64)
    sec = np.asarray(secretion, np.float64)
    act = np.asarray(active).astype(np.float64)
    D = np.asarray(diffusion_coefs, np.float32)
    K = np.asarray(degradation_rates, np.float32)

    lam = np.sqrt(D / K).astype(np.float32)          # match reference fp32 math
    lams, perm, offs, ns = _build_groups(lam)

    order = _morton_order(pos)
    inv = np.empty(N, np.int64)
    inv[order] = np.arange(N)

    ps = pos[order]
    radsq_sorted = (rad[order] ** 2).astype(np.float32)
    srcp = (sec * act[:, None] / (FOUR_PI * np.asarray(D, np.float64))[None, :])
    srcp = srcp[order][:, perm].astype(np.float32)

    centers = ps.reshape(NB, PB, 3).mean(axis=1)     # [NB, 3] f64

    # aug_src[5, N]: per block b (cols b*PB..): [x', y', z', 1, |p'|^2]
    aug_src = np.empty((5, N), np.float64)
    # aug_q per core c: [5, NB*RPC]: per block b: [-2x', -2y', -2z', |p'|^2, 1]
    aug_qs = [np.empty((5, NB * RPC), np.float64) for _ in range(NCORES)]
    for b in range(NB):
        pj = ps[b * PB:(b + 1) * PB] - centers[b]
        aug_src[0:3, b * PB:(b + 1) * PB] = pj.T
        aug_src[3, b * PB:(b + 1) * PB] = 1.0
        aug_src[4, b * PB:(b + 1) * PB] = (pj * pj).sum(1)
        for c in range(NCORES):
            pi = ps[c * RPC:(c + 1) * RPC] - centers[b]
            blk = aug_qs[c][:, b * RPC:(b + 1) * RPC]
            blk[0:3] = -2.0 * pi.T
            blk[3] = (pi * pi).sum(1)
            blk[4] = 1.0

    aug_src = aug_src.astype(np.float32)
    aug_qs = [a.astype(np.float32) for a in aug_qs]
    radsq_t = radsq_sorted.reshape(NB, PB).T.copy()              # [128, NB]
    srct = srcp.reshape(NB, PB, M).transpose(1, 0, 2).reshape(PB, NB * M).copy()

    in_maps = []
    for c in range(NCORES):
        in_maps.append({
            "aug_src": aug_src,
            "aug_q": aug_qs[c],
            "radsq": radsq_t,
            "srct": srct,
        })
    return in_maps, (lams, offs, ns), perm, order


def _get_program(groups_key):
    global _compiled
    if _compiled is not None and _compiled[0] == groups_key:
        return _compiled[1]
    nc = _build_program(*groups_key)
    _compiled = (groups_key, nc)
    return nc


def _install_ntff_hook():
    """The agent image's antenv lacks axon_hooks; recreate it so
    run_bass_kernel_spmd(trace=True) can capture NTFF profiles."""
    import sys
    import types

    if "antenv.axon_hooks" in sys.modules:
        return
    import antenv

    mod = types.ModuleType("antenv.axon_hooks")
    state = {"hook": None}
    mod.set_axon_ntff_profile_hook = lambda h: state.update(hook=h)
    mod.get_axon_ntff_profile_hook = lambda: state["hook"]
    sys.modules["antenv.axon_hooks"] = mod
    antenv.axon_hooks = mod
    try:
        from trn_agent_boot.trn_boot import _ntff_profile_via_ctypes

        mod.set_axon_ntff_profile_hook(
            _ntff_profile_via_ctypes("/opt/axon/libaxon_pjrt.so"))
    except Exception:
        pass


def _run(inputs, trace=False):
    from concourse.bass_utils import run_bass_kernel_spmd

    if trace:
        _install_ntff_hook()

    in_maps, (lams, offs, ns), perm, order = _prepare(**inputs)
    groups_key = (tuple(lams), tuple(offs), tuple(ns))
    nc = _get_program(groups_key)
    res = run_bass_kernel_spmd(nc, in_maps, core_ids=list(range(NCORES)),
                               trace=trace)
    out_sorted = np.concatenate(
        [res.results[c]["outT"].T for c in range(NCORES)], axis=0)  # [N, M] perm
    out_perm = np.empty_like(out_sorted)
    out_perm[:, perm] = out_sorted                 # undo channel permutation
    # row k of out_perm is original cell order[k]; scatter rows back
    out = np.empty_like(out_perm)
    out[order] = out_perm
    return out.astype(np.float32), res


def kernel(position, radius, secretion, diffusion_coefs, degradation_rates,
           active):
    out, _ = _run(dict(position=position, radius=radius, secretion=secretion,
                       diffusion_coefs=diffusion_coefs,
                       degradation_rates=degradation_rates, active=active))
    return out


# revision 20
# speedup vs baseline: 1.5502x; 1.5502x over previous
"""Steady-state diffusion-degradation morphogen field kernel for Trainium2.

Computes, for every cell i and morphogen m:
    conc[i, m] = sum_j G_m(r_ij) * secretion[j, m] * active[j]
with G_m(r) = exp(-r / lambda_m) / (4 pi D_m r), lambda_m = sqrt(D_m / k_m),
r_ij = max(|p_i - p_j|, radius_j).

Strategy (8 NeuronCores, data-parallel over query rows i):
  * Each core owns 512 query rows; all 4096 sources are replicated.
  * dist^2 via one K=5 augmented matmul per 128-source block:
      s[j, i] = x_j*(-2x_i) + y_j*(-2y_i) + z_j*(-2z_i) + 1*|p_i|^2 + |p_j|^2*1
    with per-source-block local centering (cells Morton-sorted) so the
    cancellation error stays at the scale of the actual distances.
  * One ACT table set (natural_log_exp) does all transcendentals:
      L = ln(max(s, radius_j^2));  r = exp(0.5*L)
      E_g = exp(-(r/lam_g + 0.5*L)) = exp(-r/lam_g)/r     (1/r folded in!)
  * Per distinct lambda the DVE builds the argument with the fused
    affine_then_add op; PE contracts E_g against stationary
    src'[j, m] = secretion*active/(4 pi D_m), accumulating in PSUM.
"""

import os
import sys

import numpy as np

for _p in ("/opt/trn_rl_repo", "/root/.axon_site/_ro/trn_rl_repo"):
    if os.path.isdir(_p) and _p not in sys.path:
        sys.path.append(_p)

N = 4096
M = 8
NCORES = 8
RPC = N // NCORES          # 512 query rows per core
PB = 128                   # source rows per block (partition dim)
NB = N // PB               # 32 source blocks
CHUNK_BLOCKS = 4           # source blocks per elementwise chunk
CHUNK_F = CHUNK_BLOCKS * RPC  # free extent of a chunk tile
FOUR_PI = 4.0 * np.pi

# dtype knobs ("f32", "f32r", "f16", "bf16")
DIST_MM_DT = "f32"         # dist^2 matmul operand dtype
REDUCE_MM_DT = "f32"      # reduction matmul operand dtype (E and src)
GPSIMD_GROUPS = 0          # GpSimd elementwise offload is a net loss (shared
                           # SBUF port starves DVE; its TENSOR_SCALAR is 17cyc/elem)

_compiled = None           # (key, nc) compile cache


def _morton_order(pos):
    """Spatial sort so each 128-cell block is spatially local."""
    span = np.maximum(pos.max(0) - pos.min(0), 1e-30)
    q = np.clip((pos - pos.min(0)) / span * 1023.0, 0, 1023).astype(np.uint64)

    def _spread(v):
        v &= 0x3FF
        v = (v | (v << 16)) & 0x030000FF
        v = (v | (v << 8)) & 0x0300F00F
        v = (v | (v << 4)) & 0x030C30C3
        v = (v | (v << 2)) & 0x09249249
        return v

    code = (_spread(q[:, 0]) << 2) | (_spread(q[:, 1]) << 1) | _spread(q[:, 2])
    return np.argsort(code, kind="stable")


def _build_groups(lam):
    """Group channels by identical fp32 lambda. Returns (lams, perm, offs, ns)."""
    uniq = np.unique(lam)
    chans, lams = [], []
    for u in uniq:
        idx = np.nonzero(lam == u)[0]
        chans.append(idx)
        lams.append(float(u))
    perm = np.concatenate(chans)
    ns = [len(c) for c in chans]
    offs = np.concatenate([[0], np.cumsum(ns)])[:-1].tolist()
    return lams, perm, offs, ns


def _patch_act_tables():
    """Keep Exp/Ln only in natural_log_exp_and_others so the table-load
    inserter picks one set for both (indices must stay act_info-aligned)."""
    from concourse import bacc, mybir

    if getattr(bacc, "_act_tables_patched", False):
        return
    orig = bacc.get_activation_tables

    def patched(arch):
        tabs = orig(arch)
        out = {}
        for name, fns in tabs.items():
            if name != "natural_log_exp_and_others":
                fns = fns - {mybir.ActivationFunctionType.Exp,
                             mybir.ActivationFunctionType.Ln}
            out[name] = fns
        return out

    bacc.get_activation_tables = patched
    bacc._act_tables_patched = True


def _build_program(group_lams, group_offs, group_ns):
    from contextlib import ExitStack

    import concourse.bass as bass
    import concourse.tile as tile
    from concourse import bacc, mybir

    _patch_act_tables()

    f32 = mybir.dt.float32
    mm_dt = {"f32": mybir.dt.float32, "f32r": mybir.dt.float32r,
             "f16": mybir.dt.float16, "bf16": mybir.dt.bfloat16}
    dist_dt = mm_dt[DIST_MM_DT]
    red_dt = mm_dt[REDUCE_MM_DT]
    Exp = mybir.ActivationFunctionType.Exp
    Ln = mybir.ActivationFunctionType.Ln

    nc = bacc.Bacc("TRN2", target_bir_lowering=False, debug=False,
                   enable_asserts=False, num_devices=NCORES)

    aug_src = nc.dram_tensor("aug_src", [5, N], f32, kind="ExternalInput").ap()
    aug_q = nc.dram_tensor("aug_q", [5, NB * RPC], f32, kind="ExternalInput").ap()
    radsq = nc.dram_tensor("radsq", [PB, NB], f32, kind="ExternalInput").ap()
    srct = nc.dram_tensor("srct", [PB, NB * M], red_dt,
                          kind="ExternalInput").ap()
    outT = nc.dram_tensor("outT", [M, RPC], f32, kind="ExternalOutput").ap()

    ngroups = len(group_lams)
    assert ngroups <= 6, "PSUM bank budget supports at most 6 lambda groups"

    with tile.TileContext(nc) as tc, ExitStack() as ctx:
        const = ctx.enter_context(tc.tile_pool(name="const", bufs=1))
        aug_src_s = const.tile([5, N], f32, tag="augsrc")
        nc.sync.dma_start(aug_src_s[:], aug_src[:])
        radsq_s = const.tile([PB, NB], f32, tag="radsq")
        nc.sync.dma_start(radsq_s[:], radsq[:])
        srct_s = const.tile([PB, NB * M], red_dt, tag="srct")
        nc.sync.dma_start(srct_s[:], srct[:])

        ps_s = ctx.enter_context(tc.tile_pool(name="ps_s", bufs=2, space="PSUM"))
        ps_o = ctx.enter_context(tc.tile_pool(name="ps_o", bufs=1, space="PSUM"))
        aq_pool = ctx.enter_context(tc.tile_pool(name="aq", bufs=4))
        sc_pool = ctx.enter_context(tc.tile_pool(name="sc", bufs=2))
        ln_pool = ctx.enter_context(tc.tile_pool(name="lnp", bufs=2))
        r_pool = ctx.enter_context(tc.tile_pool(name="rp", bufs=2))
        a_pool = ctx.enter_context(tc.tile_pool(name="ap", bufs=3))
        g_pool = ctx.enter_context(tc.tile_pool(name="gp", bufs=3))
        e_pool = ctx.enter_context(tc.tile_pool(name="ep", bufs=4))

        ps_out = [ps_o.tile([group_ns[g], RPC], f32, tag=f"out{g}",
                            name=f"ps_out{g}")
                  for g in range(ngroups)]

        nchunks = NB // CHUNK_BLOCKS

        def prologue(cc):
            """Front end of a chunk: dma + dist matmuls + clamps + ln + r."""
            sc = sc_pool.tile([PB, CHUNK_F], f32, tag="sc", name=f"sc{cc}")
            for bi in range(CHUNK_BLOCKS):
                b = cc * CHUNK_BLOCKS + bi
                aq_t = aq_pool.tile([5, RPC], f32, tag="aq", name=f"aq{b}")
                nc.sync.dma_start(aq_t[:], aug_q[:, b * RPC:(b + 1) * RPC])
                ps_tile = ps_s.tile([PB, RPC], f32, tag="s2", name=f"s2_{b}")
                nc.tensor.matmul(
                    ps_tile[:],
                    lhsT=aug_src_s[:, b * PB:(b + 1) * PB].bitcast(dist_dt),
                    rhs=aq_t[:].bitcast(dist_dt),
                    start=True, stop=True,
                )
                nc.vector.tensor_scalar_max(
                    sc[:, bi * RPC:(bi + 1) * RPC], ps_tile[:],
                    radsq_s[:, b:b + 1],
                )
            lnt = ln_pool.tile([PB, CHUNK_F], f32, tag="ln", name=f"ln{cc}")
            nc.scalar.activation(lnt[:], sc[:], Ln)
            rt = r_pool.tile([PB, CHUNK_F], f32, tag="r", name=f"r{cc}")
            nc.scalar.activation(rt[:], lnt[:], Exp, scale=0.5)
            return lnt, rt

        def body(cc, lnt, rt):
            """Per-group argument build, exp, and reduction matmuls."""
            for g in range(ngroups):
                lam_g = group_lams[g]
                at = a_pool.tile([PB, CHUNK_F], f32, tag="a", name=f"a{cc}_{g}")
                nc.vector.affine_then_add(
                    at[:], in0=lnt[:], in1=rt[:], scale=lam_g * 0.5, bias=0.0)
                et = e_pool.tile([PB, CHUNK_F], red_dt, tag="e",
                                 name=f"e{cc}_{g}")
                nc.scalar.activation(et[:], at[:], Exp, scale=-1.0 / lam_g)
                for bi in range(CHUNK_BLOCKS):
                    b = cc * CHUNK_BLOCKS + bi
                    nc.tensor.matmul(
                        ps_out[g][:],
                        lhsT=srct_s[:, b * M + group_offs[g]:
                                    b * M + group_offs[g] + group_ns[g]],
                        rhs=et[:, bi * RPC:(bi + 1) * RPC],
                        start=(b == 0), stop=(b == NB - 1),
                    )

        # software-pipelined emission: next chunk's front end is issued
        # before this chunk's group work so the in-order engines never
        # queue group work ahead of the front end that gates the pipeline
        pending = prologue(0)
        for cc in range(nchunks):
            nxt = prologue(cc + 1) if cc + 1 < nchunks else None
            body(cc, *pending)
            pending = nxt

        out_pool = ctx.enter_context(tc.tile_pool(name="outp", bufs=2))
        for g in range(ngroups):
            o = group_offs[g]
            sb = out_pool.tile([3, RPC], f32, tag="osb", name=f"out_sb{g}")
            nc.vector.tensor_copy(sb[0:group_ns[g], :], ps_out[g][:])
            nc.sync.dma_start(outT[o:o + group_ns[g], :], sb[0:group_ns[g], :])

    nc.compile()
    return nc


def _prepare(position, radius, secretion, diffusion_coefs, degradation_rates,
             active):
    pos = np.asarray(position, np.float64)
    rad = np.asarray(radius, np.float64)
    sec = np.asarray(secretion, np.float64)
    act = np.asarray(active).astype(np.float64)
    D = np.asarray(diffusion_coefs, np.float32)
    K = np.asarray(degradation_rates, np.float32)

    lam = np.sqrt(D / K).astype(np.float32)          # match reference fp32 math
    lams, perm, offs, ns = _build_groups(lam)

    order = _morton_order(pos)
    inv = np.empty(N, np.int64)
    inv[order] = np.arange(N)

    ps = pos[order]
    radsq_sorted = (rad[order] ** 2).astype(np.float32)
    srcp = (sec * act[:, None] / (FOUR_PI * np.asarray(D, np.float64))[None, :])
    srcp = srcp[order][:, perm].astype(np.float32)

    centers = ps.reshape(NB, PB, 3).mean(axis=1)     # [NB, 3] f64

    # aug_src[5, N]: per block b (cols b*PB..): [x', y', z', 1, |p'|^2]
    aug_src = np.empty((5, N), np.float64)
    # aug_q per core c: [5, NB*RPC]: per block b: [-2x', -2y', -2z', |p'|^2, 1]
    aug_qs = [np.empty((5, NB * RPC), np.float64) for _ in range(NCORES)]
    for b in range(NB):
        pj = ps[b * PB:(b + 1) * PB] - centers[b]
        aug_src[0:3, b * PB:(b + 1) * PB] = pj.T
        aug_src[3, b * PB:(b + 1) * PB] = 1.0
        aug_src[4, b * PB:(b + 1) * PB] = (pj * pj).sum(1)
        for c in range(NCORES):
            pi = ps[c * RPC:(c + 1) * RPC] - centers[b]
            blk = aug_qs[c][:, b * RPC:(b + 1) * RPC]
            blk[0:3] = -2.0 * pi.T
            blk[3] = (pi * pi).sum(1)
            blk[4] = 1.0

    aug_src = aug_src.astype(np.float32)
    aug_qs = [a.astype(np.float32) for a in aug_qs]
    radsq_t = radsq_sorted.reshape(NB, PB).T.copy()              # [128, NB]
    srct = srcp.reshape(NB, PB, M).transpose(1, 0, 2).reshape(PB, NB * M).copy()

    in_maps = []
    for c in range(NCORES):
        in_maps.append({
            "aug_src": aug_src,
            "aug_q": aug_qs[c],
            "radsq": radsq_t,
            "srct": srct,
        })
    return in_maps, (lams, offs, ns), perm, order


def _get_program(groups_key):
    global _compiled
    if _compiled is not None and _compiled[0] == groups_key:
        return _compiled[1]
    nc = _build_program(*groups_key)
    _compiled = (groups_key, nc)
    return nc


def _install_ntff_hook():
    """The agent image's antenv lacks axon_hooks; recreate it so
    run_bass_kernel_spmd(trace=True) can capture NTFF profiles."""
    import sys
    import types

    if "antenv.axon_hooks" in sys.modules:
        return
    import antenv

    mod = types.ModuleType("antenv.axon_hooks")
    state = {"hook": None}
    mod.set_axon_ntff_profile_hook = lambda h: state.update(hook=h)
    mod.get_axon_ntff_profile_hook = lambda: state["hook"]
    sys.modules["antenv.axon_hooks"] = mod
    antenv.axon_hooks = mod
    try:
        from trn_agent_boot.trn_boot import _ntff_profile_via_ctypes

        mod.set_axon_ntff_profile_hook(
            _ntff_profile_via_ctypes("/opt/axon/libaxon_pjrt.so"))
    except Exception:
        pass


def _run(inputs, trace=False):
    from concourse.bass_utils import run_bass_kernel_spmd

    if trace:
        _install_ntff_hook()

    in_maps, (lams, offs, ns), perm, order = _prepare(**inputs)
    groups_key = (tuple(lams), tuple(offs), tuple(ns))
    nc = _get_program(groups_key)
    res = run_bass_kernel_spmd(nc, in_maps, core_ids=list(range(NCORES)),
                               trace=trace)
    out_sorted = np.concatenate(
        [res.results[c]["outT"].T for c in range(NCORES)], axis=0)  # [N, M] perm
    out_perm = np.empty_like(out_sorted)
    out_perm[:, perm] = out_sorted                 # undo channel permutation
    # row k of out_perm is original cell order[k]; scatter rows back
    out = np.empty_like(out_perm)
    out[order] = out_perm
    return out.astype(np.float32), res


def kernel(position, radius, secretion, diffusion_coefs, degradation_rates,
           active):
    out, _ = _run(dict(position=position, radius=radius, secretion=secretion,
                       diffusion_coefs=diffusion_coefs,
                       degradation_rates=degradation_rates, active=active))
    return out
